# revision 42
# baseline (speedup 1.0000x reference)
"""AdaptivePoolingAttention on 8 TRN2 NeuronCores.

Data-parallel over segments: each core owns 4 of the 32 segments (attention is
block-diagonal per segment), weights replicated. No collectives.

Fused single-pass pipeline (v3). All matmuls bf16 with fp32 PSUM accumulation.
v is computed directly in NATURAL [token, hd] layout (stationary = xT chunk,
moving = wkv columns) so there is no DRAM kv spill and no transposed DMA
re-read. k is produced transposed (kT) per head; attention follows each head
pair immediately: scores are computed transposed (sT [tok, q]) so the softmax
denominator comes from a ones-column in the attn@v matmul, and o is normalized
after PSUM accumulation with a per-partition reciprocal, then transposed once
for the output projection. Work is emitted as uniform interleaved phases —
phase(s) = k+attention(s) + v(s+1) + extras — so tensor-engine work and weight
DMA stay evenly paced:
  prologue:  v(s0) + pooling + query rmsnorm
  phase(0):  k+attn(s0) + v(s1) + q-projection (wq streams here)
  phase(1):  k+attn(s1) + v(s2)
  phase(2):  k+attn(s2) + v(s3) + out-projection half 0 (wo stream 1)
  phase(3):  k+attn(s3)
  tail:      out-projection half 1 (wo stream 2) + final rmsnorm

Host wrapper pre-packs layouts (pure data movement + bf16 rounding);
attn_norm_w is folded into wq (rmsnorm(q)*w @ wq == rmsnorm(q) @ diag(w)wq).
  xT       [D, TOK]  bf16 per core (x transposed)
  wq_pack  [H, 128, D]      bf16: wq_pack[h, p, dd*128+c] = wq'[dd*128+p, h*128+c]
  wkv_pack [KV/128, 128, D] bf16: same per 128-col tile
  wo_pack  [D/NJ, 128, H*NJ] bf16: wo_pack[j, p, hh*NJ+c] = wo[hh*128+p, j*NJ+c]
"""

import sys

sys.path.insert(0, "/opt/trn_rl_repo")

import numpy as np
from contextlib import ExitStack

SEG, L, R, POOL, H, HD, D, EPS = 32, 512, 64, 8, 32, 128, 4096, 1e-5
NCORES = 8
SEGC = SEG // NCORES          # 4 segments per core
TOK = SEGC * L                # 2048 tokens per core
Q = SEGC * R                  # 256 queries per core
KV = 2 * H * HD               # 8192
NJ = 128                      # wo col-tile width
ND = D // 128                 # 32 contraction tiles

_CACHE = {}


def _build():
    import concourse.bass as bass
    import concourse.mybir as mybir
    import concourse.tile as tile
    from concourse import bacc

    f32 = mybir.dt.float32
    bf16 = mybir.dt.bfloat16
    ts = bass.ts
    ds = bass.ds
    AF = mybir.ActivationFunctionType
    ALU = mybir.AluOpType

    nc = bacc.Bacc("TRN2", target_bir_lowering=False, debug=False)

    xT_e = nc.declare_dram_parameter("xT", [D, TOK], bf16, isOutput=False)
    wq_e = nc.declare_dram_parameter("wq_pack", [H, 128, D], bf16, isOutput=False)
    wkv_e = nc.declare_dram_parameter(
        "wkv_pack", [KV // 128, 128, D], bf16, isOutput=False
    )
    wo_e = nc.declare_dram_parameter(
        "wo_pack", [D // NJ, 128, H * NJ], bf16, isOutput=False
    )
    onw_e = nc.declare_dram_parameter("out_norm_w", [128, D], bf16, isOutput=False)
    id_e = nc.declare_dram_parameter("ident", [128, 128], bf16, isOutput=False)
    out_e = nc.declare_dram_parameter("out", [Q, D], f32, isOutput=True)

    qspill_d = nc.dram_tensor("q_spill", [2, 128, D], bf16)       # 2 MiB
    oscr_d = nc.dram_tensor("out_scratch", [2, 128, D], bf16)     # 2 MiB

    with tile.TileContext(nc, pool_alloc_mode="queue") as tc, ExitStack() as st:
        # ---- constants ------------------------------------------------
        cst = st.enter_context(tc.tile_pool(name="const", bufs=1))
        ident = cst.tile([128, 128], bf16)
        epst = cst.tile([128, 1], f32)
        nc.vector.memset(epst[:], EPS)
        nc.sync.dma_start(ident[:], id_e[:])

        # persistent tiles
        qTp = st.enter_context(tc.tile_pool(name="qTp", bufs=1))
        qT = qTp.tile([128, H, Q], bf16)          # 16 KiB
        vNp = st.enter_context(tc.tile_pool(name="vNp", bufs=1))
        vN = vNp.tile([128, 4, H, 130], bf16)     # 32.5 KiB, reused per segment
        xTp = st.enter_context(tc.tile_pool(name="xTp", bufs=2))
        wsp = st.enter_context(tc.tile_pool(name="wsp", bufs=3))

        # v and k accumulation chains share one PSUM bank-pool (the tensor
        # engine runs them in order anyway); v uses the first 256 cols only
        chn_p = st.enter_context(tc.tile_pool(name="chn", bufs=3, space="PSUM"))

        xTs_cur = [None]  # current segment's xT tile

        def load_xT(s, xTs):
            for dblk in range(ND):
                nc.sync.dma_start(
                    xTs[:, dblk, :], xT_e[ds(dblk * 128, 128), ts(s, 512)]
                )

        def v_proj_hp(xTs, hp, split=False):
            """v for head-pair hp, natural [token, hd] layout, into vN."""
            wv2 = wsp.tile([128, ND, 256], bf16, tag="wv2", name="wv2")
            for dh in range(2 if split else 1):
                for j in range(2):
                    sl = ds(dh * (ND // 2), ND // 2) if split else slice(None)
                    nc.scalar.dma_start(
                        wv2[:, sl, ts(j, 128)],
                        wkv_e[H + 2 * hp + j]
                        .rearrange("p (dd c) -> p dd c", c=128)[:, sl, :],
                    )
            for tc_ in range(4):
                vch = chn_p.tile([128, 512], f32, tag="mm", name="vch")
                vps = vch[:, 0:256]
                for dblk in range(ND):
                    nc.tensor.matmul(
                        vps, xTs[:, dblk, ts(tc_, 128)], wv2[:, dblk, :],
                        start=(dblk == 0), stop=(dblk == ND - 1),
                    )
                for j in range(2):
                    nc.any.tensor_copy(
                        vN[:, tc_, 2 * hp + j, 0:128], vch[:, ts(j, 128)]
                    )

        # ======== prologue: v(s0) + pooled queries + rmsnorm ============
        xT0 = xTp.tile([128, ND, 512], bf16, tag="xT", name="xT0")
        load_xT(0, xT0)
        xTs_cur[0] = xT0
        nc.vector.memset(vN[:, :, :, 128:129], 1.0)
        v_proj_hp(xT0, 0, split=True)
        v_proj_hp(xT0, 1)

        qq_es = ExitStack()
        qq_p = qq_es.enter_context(tc.tile_pool(name="qq", bufs=1, side="right"))
        qnT_holder = [None]

        with tc.tile_pool(name="pstr", bufs=2) as pstr_p, \
             tc.tile_pool(name="qn", bufs=1) as qn_p, \
             tc.tile_pool(name="ptr", bufs=2, space="PSUM") as ptr_p:
            q_nat = [
                qn_p.tile([128, D], bf16, tag=f"qnat{i}", name=f"qnat{i}")
                for i in range(2)
            ]
            # qpT and qnT time-share one region (qpT fully consumed before
            # qnT is written; bufs=1 rotation orders the reuse)
            qpT = qq_p.tile([128, ND, Q], bf16, tag="qq", name="qpT")

            def pool_chunks(dblks):
                # pooling: stream xT by d-block halves on the sync queue
                for dblk in dblks:
                    for hf in range(2):
                        pstr = pstr_p.tile(
                            [128, TOK // 2], bf16, tag="ps", name="ps"
                        )
                        nc.sync.dma_start(
                            pstr[:],
                            xT_e[ds(dblk * 128, 128), ts(hf, TOK // 2)],
                        )
                        qtrf = pstr_p.tile(
                            [128, Q // 2], f32, tag="qf", name="qf"
                        )
                        nc.vector.tensor_reduce(
                            qtrf[:],
                            pstr[:].rearrange("p (q e) -> p q e", e=POOL),
                            axis=mybir.AxisListType.X, op=ALU.add,
                        )
                        nc.vector.tensor_scalar_mul(
                            qpT[:, dblk, ts(hf, Q // 2)], qtrf[:], 1.0 / POOL
                        )

            def pool_T(qt, dblks):
                for dblk in dblks:
                    ptr = ptr_p.tile([128, 128], bf16, tag="ptr")
                    nc.tensor.transpose(
                        ptr[:], qpT[:, dblk, ts(qt, 128)], ident[:]
                    )
                    nc.any.tensor_copy(q_nat[qt][:, ts(dblk, 128)], ptr[:])

            def b_norm(qt):
                # spill raw queries (residual), then rmsnorm q_nat in place
                nc.sync.dma_start(qspill_d[qt], q_nat[qt][:])
                ssq = qn_p.tile([128, 1], f32, tag="ssq", name="ssq")
                ssqa = qn_p.tile([128, 1], f32, tag="ssqa", name="ssqa")
                scr = qn_p.tile([128, D // 2], bf16, tag="scr", name="scr")
                for half in range(2):
                    nc.scalar.activation(
                        scr[:], q_nat[qt][:, ts(half, D // 2)],
                        AF.Square, accum_out=(ssq if half else ssqa)[:],
                    )
                nc.vector.tensor_tensor(ssq[:], ssq[:], ssqa[:], op=ALU.add)
                srt = qn_p.tile([128, 1], f32, tag="srt", name="srt")
                nc.scalar.activation(
                    srt[:], ssq[:], AF.Sqrt, bias=epst[:], scale=1.0 / D
                )
                rs = qn_p.tile([128, 1], f32, tag="rs", name="rs")
                nc.vector.reciprocal(rs[:], srt[:])
                nc.vector.tensor_scalar_mul(q_nat[qt][:], q_nat[qt][:], rs[:])

            def b_T(qt, qnT):
                for dblk in range(ND):
                    ptr = ptr_p.tile([128, 128], bf16, tag="ptr")
                    nc.tensor.transpose(
                        ptr[:], q_nat[qt][:, ts(dblk, 128)], ident[:]
                    )
                    nc.any.tensor_copy(qnT[:, dblk, ts(qt, 128)], ptr[:])

            pool_chunks(range(0, 12))
            for hp in range(2, 16):
                v_proj_hp(xT0, hp)
                if hp == 2:
                    pool_chunks(range(12, 24))
                elif hp == 3:
                    pool_chunks(range(24, 32))
                    pool_T(0, range(0, 12))
                elif hp == 4:
                    pool_T(0, range(12, 24))
                elif hp == 5:
                    pool_T(0, range(24, 32))
                    pool_T(1, range(0, 8))
                elif hp == 6:
                    pool_T(1, range(8, 32))
                    b_norm(0)
                elif hp == 7:
                    qnT_holder[0] = qq_p.tile(
                        [128, ND, Q], bf16, tag="qq", name="qnT"
                    )
                    b_T(0, qnT_holder[0])
                elif hp == 8:
                    b_norm(1)
                elif hp == 9:
                    b_T(1, qnT_holder[0])

        qnT = qnT_holder[0]

        # ======== phases: k+attention(s) + v(s+1) + projections =========
        with tc.tile_pool(name="oTp", bufs=1) as oTp, \
             tc.tile_pool(name="kTp", bufs=2) as kTp, \
             tc.tile_pool(name="att", bufs=4) as att_p, \
             tc.tile_pool(name="aps", bufs=1, space="PSUM") as aps_p:
            oT = oTp.tile([128, H, Q], bf16)      # 16 KiB

            def k_pair(h0):
                # one 2-head weight tile -> two kT tiles
                wkb = wsp.tile([128, ND, 256], bf16, tag="wv2", name="wkb")
                for j in range(2):
                    nc.sync.dma_start(
                        wkb[:, :, ts(j, 128)],
                        wkv_e[h0 + j].rearrange("p (dd c) -> p dd c", c=128),
                    )
                kts = []
                for j in range(2):
                    kps = chn_p.tile([128, 512], f32, tag="mm", name="kch")
                    for dblk in range(ND):
                        nc.tensor.matmul(
                            kps[:], wkb[:, dblk, ts(j, 128)],
                            xTs_cur[0][:, dblk, :],
                            start=(dblk == 0), stop=(dblk == ND - 1),
                        )
                    kTh = kTp.tile([128, 512], bf16, tag="kT", name="kT")
                    nc.any.tensor_copy(kTh[:], kps[:])
                    kts.append(kTh)
                return kts

            def c_pair(hp):
                # q-projection pair: qT = (wq^T @ qnT) * HD**-0.5
                wqb = wsp.tile([128, ND, 256], bf16, tag="wv2", name="wqb")
                for j in range(2):
                    nc.scalar.dma_start(
                        wqb[:, :, ts(j, 128)],
                        wq_e[2 * hp + j].rearrange("p (dd c) -> p dd c", c=128),
                    )
                for j in range(2):
                    psq = aps_p.tile([128, Q], f32, tag="uc", name="cps")
                    for dblk in range(ND):
                        nc.tensor.matmul(
                            psq[:], wqb[:, dblk, ts(j, 128)], qnT[:, dblk, :],
                            start=(dblk == 0), stop=(dblk == ND - 1),
                        )
                    nc.scalar.mul(
                        qT[:, 2 * hp + j, :], psq[:], float(HD) ** -0.5
                    )

            def attn_sT(s, p, kTa, kTb):
                sps = aps_p.tile([128, 8, 64], f32, tag="s", name="s")
                exps = []
                for hi, kTh in ((0, kTa), (1, kTb)):
                    h = 2 * p + hi
                    for tc_ in range(4):
                        nc.tensor.matmul(
                            sps[:, hi * 4 + tc_, :], kTh[:, ts(tc_, 128)],
                            qT[:, h, ds(s * R, R)],
                            start=True, stop=True,
                        )
                    ex = att_p.tile([128, 4, 64], bf16, tag="exp", name="exp")
                    nc.scalar.activation(
                        ex[:], sps[:, ds(hi * 4, 4), :], AF.Exp
                    )
                    exps.append(ex)
                return exps

            def attn_u(s, p, exps):
                ups = aps_p.tile([128, Q], f32, tag="uc", name="u")
                for hi in range(2):
                    h = 2 * p + hi
                    for tc_ in range(4):
                        nc.tensor.matmul(
                            ups[ds(hi * 64, 64), 0:129],
                            exps[hi][:, tc_, :], vN[:, tc_, h, 0:129],
                            start=(tc_ == 0), stop=(tc_ == 3),
                            tile_position=(0, hi * 64),
                        )
                return ups

            def attn_o(s, p, ups):
                r = att_p.tile([128, 1], f32, tag="r", name="r")
                nc.vector.reciprocal(r[:], ups[:, 128:129])
                onat = att_p.tile([128, 128], bf16, tag="onat", name="onat")
                nc.vector.tensor_scalar_mul(onat[:], ups[:, 0:128], r[:])
                pt = aps_p.tile([128, 128], bf16, tag="t", name="t")
                nc.tensor.transpose(pt[:], onat[:], ident[:])
                nc.any.tensor_copy(
                    oT[:, ds(2 * p, 2), ds(s * R, R)],
                    pt[:].rearrange("p (a b) -> p a b", a=2),
                )

            f_es = ExitStack()
            fpools = {}

            def open_f_pools():
                fpools["onwp"] = f_es.enter_context(
                    tc.tile_pool(name="onwp", bufs=1))
                fpools["wob"] = f_es.enter_context(
                    tc.tile_pool(name="wob", bufs=3))
                fpools["och"] = f_es.enter_context(
                    tc.tile_pool(name="och", bufs=2, side="right"))
                fpools["qrl"] = f_es.enter_context(
                    tc.tile_pool(name="qrl", bufs=2))
                fpools["fsq"] = f_es.enter_context(
                    tc.tile_pool(name="fsq", bufs=1))
                fpools["fns"] = f_es.enter_context(
                    tc.tile_pool(name="fns", bufs=2))
                fpools["fps"] = f_es.enter_context(
                    tc.tile_pool(name="fps", bufs=2, space="PSUM"))
                fpools["ssqs"] = fpools["fsq"].tile(
                    [128, 2, D // NJ], f32, tag="ssqs", name="ssqs")
                fpools["onw"] = fpools["onwp"].tile(
                    [128, D], bf16, tag="onw", name="onw")
                nc.sync.dma_start(fpools["onw"][:], onw_e[:])

            with ExitStack() as f_es_guard:
                f_es_guard.enter_context(f_es)

                def f_block(qt, j):
                    wob_p = fpools["wob"]
                    och_p = fpools["och"]
                    qrl_p = fpools["qrl"]
                    fps_p = fpools["fps"]
                    ssqs = fpools["ssqs"]
                    smq = nc.scalar if qt else nc.sync
                    wobs = []
                    for hf in range(2):
                        wob = wob_p.tile([128, H // 2, NJ], bf16, tag="wob",
                                         name="wob")
                        nc.sync.dma_start(
                            wob[:],
                            wo_e[j, :, ds(hf * (H // 2) * NJ, (H // 2) * NJ)]
                            .rearrange("p (hh c) -> p hh c", c=NJ),
                        )
                        wobs.append(wob)
                    ps = fps_p.tile([128, NJ], f32)
                    for hh in range(H):
                        nc.tensor.matmul(
                            ps[:], oT[:, hh, ts(qt, 128)],
                            wobs[hh // (H // 2)][:, hh % (H // 2), :],
                            start=(hh == 0), stop=(hh == H - 1),
                        )
                    qrl = qrl_p.tile([128, NJ], bf16, tag="q", name="q")
                    smq.dma_start(qrl[:], qspill_d[qt][:, ts(j, NJ)])
                    och = och_p.tile([128, NJ], bf16, tag="och", name="och")
                    nc.vector.tensor_tensor(och[:], ps[:], qrl[:], op=ALU.add)
                    smq.dma_start(oscr_d[qt][:, ts(j, NJ)], och[:])
                    sq = och_p.tile([128, NJ], bf16, tag="sq", name="sq")
                    nc.scalar.activation(
                        sq[:], och[:], AF.Square,
                        accum_out=ssqs[:, qt, ds(j, 1)],
                    )

                def f_norm(qt):
                    fns_p = fpools["fns"]
                    ssqs = fpools["ssqs"]
                    onw_res = fpools["onw"]
                    smq = nc.scalar if qt else nc.sync
                    ssq = fns_p.tile([128, 1], f32, tag="fssq")
                    nc.vector.tensor_reduce(
                        ssq[:], ssqs[:, qt, :],
                        axis=mybir.AxisListType.X, op=ALU.add,
                    )
                    srt = fns_p.tile([128, 1], f32, tag="fsrt")
                    nc.scalar.activation(
                        srt[:], ssq[:], AF.Sqrt, bias=epst[:], scale=1.0 / D
                    )
                    rs = fns_p.tile([128, 1], f32, tag="frs")
                    nc.vector.reciprocal(rs[:], srt[:])
                    NQ = D // 32
                    for qtr in range(32):
                        rdt = fns_p.tile([128, NQ], bf16, tag="rd")
                        smq.dma_start(rdt[:], oscr_d[qt][:, ts(qtr, NQ)])
                        fin = fns_p.tile([128, NQ], f32, tag="fin")
                        nc.vector.tensor_scalar_mul(fin[:], rdt[:], rs[:])
                        nc.vector.tensor_tensor(
                            fin[:], fin[:], onw_res[:, ts(qtr, NQ)], op=ALU.mult
                        )
                        nc.sync.dma_start(out_e[ts(qt, 128), ts(qtr, NQ)], fin[:])

                def phase(s, xTs_next):
                    if s == 0:
                        c_pair(0)
                        c_pair(1)
                    exps_q, ups_q = {}, {}
                    for p in range(16):
                        if p == 1 and xTs_next is not None:
                            load_xT(s + 1, xTs_next)
                        kTa, kTb = k_pair(2 * p)
                        exps_q[p] = attn_sT(s, p, kTa, kTb)
                        if p - 1 in exps_q:
                            ups_q[p - 1] = attn_u(s, p - 1, exps_q.pop(p - 1))
                        if p - 2 in ups_q:
                            attn_o(s, p - 2, ups_q.pop(p - 2))
                        if xTs_next is not None and p >= 2:
                            v_proj_hp(xTs_next, p - 2)
                        if s == 0 and p <= 13:
                            c_pair(p + 2)
                        if s == 2:
                            f_block(0, 2 * p)
                            f_block(0, 2 * p + 1)
                    ups_q[15] = attn_u(s, 15, exps_q.pop(15))
                    attn_o(s, 14, ups_q.pop(14))
                    attn_o(s, 15, ups_q.pop(15))
                    if xTs_next is not None:
                        v_proj_hp(xTs_next, 14)
                        v_proj_hp(xTs_next, 15)

                for s in range(SEGC):
                    xTs_next = (
                        xTp.tile([128, ND, 512], bf16, tag="xT",
                                 name=f"xT{s + 1}")
                        if s < SEGC - 1 else None
                    )
                    phase(s, xTs_next)
                    if s == 0:
                        qq_es.close()
                        open_f_pools()
                    if s == 2:
                        f_norm(0)
                    if xTs_next is not None:
                        xTs_cur[0] = xTs_next

                # tail: second out-projection half + final norm
                for j in range(D // NJ):
                    f_block(1, j)
                f_norm(1)

    nc.finalize()
    return nc


def _in_maps(inputs):
    import ml_dtypes

    bf = ml_dtypes.bfloat16
    x = np.asarray(inputs["x"], dtype=np.float32)
    wq = np.asarray(inputs["wq"], dtype=np.float32)
    wkv = np.asarray(inputs["wkv"], dtype=np.float32)
    wo = np.asarray(inputs["wo"], dtype=np.float32)
    anw = np.asarray(inputs["attn_norm_w"], dtype=np.float32)

    # layout packs (host-side data movement + bf16 rounding)
    xT = np.ascontiguousarray(x.T.astype(bf))                       # [D, 16384]
    # attn_norm_w folds into wq: (qn*w) @ wq == qn @ (diag(w) @ wq)
    wq_eff = anw[:, None] * wq
    # wq_pack[h, p, dd*128+c] = wq_eff[dd*128+p, h*128+c]
    wq_pack = np.ascontiguousarray(
        wq_eff.astype(bf).reshape(ND, 128, H, 128).transpose(2, 1, 0, 3).reshape(
            H, 128, D
        )
    )
    wkv_pack = np.ascontiguousarray(
        wkv.astype(bf).reshape(ND, 128, KV // 128, 128)
        .transpose(2, 1, 0, 3).reshape(KV // 128, 128, D)
    )
    # wo_pack[j, p, hh*NJ+c] = wo[hh*128+p, j*NJ+c]
    wo_pack = np.ascontiguousarray(
        wo.astype(bf).reshape(H, 128, D // NJ, NJ).transpose(2, 1, 0, 3).reshape(
            D // NJ, 128, H * NJ
        )
    )
    onw = np.ascontiguousarray(
        np.broadcast_to(
            np.asarray(inputs["out_norm_w"], dtype=np.float32).reshape(1, D),
            (128, D),
        ).astype(bf)
    )
    ident = np.eye(128, dtype=np.float32).astype(bf)
    return [
        {
            "xT": np.ascontiguousarray(xT[:, i * TOK : (i + 1) * TOK]),
            "wq_pack": wq_pack,
            "wkv_pack": wkv_pack,
            "wo_pack": wo_pack,
            "out_norm_w": onw,
            "ident": ident,
        }
        for i in range(NCORES)
    ]


def kernel(**inputs):
    from concourse.bass_utils import run_bass_kernel_spmd

    if "nc" not in _CACHE:
        _CACHE["nc"] = _build()
    nc = _CACHE["nc"]
    res = run_bass_kernel_spmd(nc, _in_maps(inputs), core_ids=list(range(NCORES)))
    out = np.concatenate(
        [res.results[i]["out"] for i in range(NCORES)], axis=0
    ).astype(np.float32)
    return out
